# revision 5
# baseline (speedup 1.0000x reference)
"""Trainium2 Bass kernel for GPUTimeMask: zero out per-batch time windows.

Semantics (matches reference):
    out = x.copy();  for m, b:  out[b, :, s[m,b] : s[m,b]+clip(w[m,b],1,150)] = 0

Strategy (v4 — DRAM->DRAM streaming on three queues):
  - Shard x along the CHANNEL axis: 16 channels -> 2 per core across 8 cores.
    Every core holds ALL 64 batch rows, so the (runtime-valued) mask windows
    live at identical local coordinates on every core -> one SPMD program
    with window offsets specialized in at build time.
  - Per core the output is a byte-for-byte copy of the input except ~128
    tiny windows (<= 2 rows x 150 cols).  Instead of staging through SBUF
    (which pins the kernel to the ~435 GB/s SBUF-AXI fabric ceiling), issue
    big DRAM->DRAM DMA copies: each SDMA descriptor reads and writes HBM
    inline, so HBM runs duplex (~650 GB/s measured; the ~330 GB/s copy rate
    is the 16 SDMA engines' read->write turnaround, insensitive to
    descriptor size and queue count) with no SBUF pipeline, no WARs, and no
    compute engines in the path.  D2D moves each byte with ONE descriptor
    instead of two, so it is ~1.3x more SDMA-engine-efficient than any
    SBUF-staged scheme.
  - The plane [128, 60000] f32 is split into contiguous row groups spread
    over THREE issue queues: qSP + qAct (HWDGE) and the gpsimd SWDGE queue.
    gpsimd's groups all sit EARLY in the stream: its ~0.84 us/instruction
    fill issue rate makes it the slowest filler, so it must finish its
    groups (copies + fills) while the HWDGE queues still stream.  The
    HWDGE queues carry the late groups and end with a tiny 2-row group
    whose ~2 fills are the only work left after the last copy lands.
  - Mask windows are overwritten with zeros by tiny DMAs sourced from a
    memset SBUF tile.  Each queue fills the windows of its OWN groups,
    interleaved behind its copy stream with a 3-group lag, so the copy-
    completion waits in front of each fill run are already satisfied when
    the sequencer reaches them and no queue builds a fill backlog.
  - Raw bass (no TileContext): semaphores placed by hand, one wait per
    instruction, and the only end-of-kernel cost is the terminal waits.
  - Programs are cached keyed on (starts, widths) bytes.
"""

import sys

import numpy as np

for _p in ("/opt/trn_rl_repo",):
    if _p not in sys.path:
        sys.path.insert(0, _p)

import concourse.bass as bass
import concourse.mybir as mybir
from concourse.bass_utils import run_bass_kernel_spmd

B, C, T = 64, 16, 60000
MAX_MASK_WIDTH = 150
N_CORES = 8
C_LOCAL = C // N_CORES          # 2 channels per core
P = B * C_LOCAL                 # 128 rows: row = b * C_LOCAL + c_local

ROW_BYTES = T * 4
PRIME = 3                       # copies enqueued ahead of the first fill wait

# Row-group schedule.  Each entry is (queue, n_rows); rows are assigned in
# order.  Queues: 0 = qSP, 1 = qAct, 2 = gpsimd SWDGE.  gpsimd gets 8
# early 4-row groups; SP/Act alternate through the rest and finish on one
# 2-row group each so the post-copy fill tail is ~2 fills.
_SCHEDULE: list[tuple[int, int]] = []
_gp_left = 8
_rows_left = P
_turn = 0
while _rows_left > 0:
    q = (0, 1, 2)[_turn % 3] if _gp_left > 0 else (0, 1)[_turn % 2]
    if q == 2:
        _gp_left -= 1
    if _rows_left <= 4:
        # terminal 2-row groups on the HWDGE queues
        _SCHEDULE.append((0, 2))
        _SCHEDULE.append((1, 2))
        _rows_left -= 4
    else:
        n = 4 if _rows_left > 8 else 2
        _SCHEDULE.append((q, n))
        _rows_left -= n
    _turn += 1
assert sum(n for _, n in _SCHEDULE) == P

_program_cache: dict[bytes, bass.Bass] = {}


def _merged_windows(starts: np.ndarray, widths: np.ndarray) -> list[list[tuple[int, int]]]:
    """Per-batch union of mask intervals (merge overlapping/adjacent)."""
    w = np.clip(widths, 1, MAX_MASK_WIDTH)
    out: list[list[tuple[int, int]]] = []
    for b in range(B):
        ivs = sorted(
            (int(starts[m, b]), min(int(starts[m, b]) + int(w[m, b]), T))
            for m in range(starts.shape[0])
        )
        merged = [ivs[0]]
        for s, e in ivs[1:]:
            if s <= merged[-1][1]:
                merged[-1] = (merged[-1][0], max(merged[-1][1], e))
            else:
                merged.append((s, e))
        out.append([(s, e) for s, e in merged if s < e])
    return out


def _build_program(windows: list[list[tuple[int, int]]]) -> bass.Bass:
    """windows[b]: merged (lo, hi) column ranges to zero; identical per core."""
    nc = bass.Bass()
    x = nc.declare_dram_parameter("x", [P, T], mybir.dt.float32, isOutput=False)
    y = nc.declare_dram_parameter("y", [P, T], mybir.dt.float32, isOutput=True)

    # groups[k] = (queue, r0, r1) in row order; per-queue ordered lists
    groups = []
    r = 0
    for q, n in _SCHEDULE:
        groups.append((q, r, r + n))
        r += n
    qgroups = [[k for k, (q, _, _) in enumerate(groups) if q == qq] for qq in range(3)]

    copy_sems = [nc.alloc_semaphore(f"copy_g{k}") for k in range(len(groups))]
    fill_sems = [nc.alloc_semaphore(f"fills_q{q}") for q in range(3)]
    engines = [nc.sync, nc.scalar, nc.gpsimd]

    def group_fills(k):
        _, r0, r1 = groups[k]
        out = []
        for b in range(r0 // C_LOCAL, r1 // C_LOCAL):
            for lo, hi in windows[b]:
                out.append((b * C_LOCAL, lo, hi))
        return out

    with nc.sbuf_tensor("zeros", [32, MAX_MASK_WIDTH + 2], mybir.dt.float32) as zsb:
        # Zero source for the window fills.  gpsimd memsets it and bumps
        # each queue's fill sem once; the HWDGE queues check the handshake
        # right before their FIRST fill (long satisfied by then), keeping
        # it off the copy-issue path.
        nc.gpsimd.memset(zsb[:], 0.0)
        for q in range(3):
            nc.gpsimd.sem_inc(fill_sems[q], 1)

        n_fills_q = [0] * 3

        for q, eng in enumerate(engines):
            gs = qgroups[q]

            def copy(k):
                _, r0, r1 = groups[k]
                eng.dma_start(
                    out=y[r0:r1, :], in_=x[r0:r1, :], max_dma_last_dim=30720
                ).then_inc(copy_sems[k], 16)

            def fill(k, first):
                if first and q != 2:
                    eng.wait_ge(fill_sems[q], 1)
                eng.wait_ge(copy_sems[k], 16)
                for row, lo, hi in group_fills(k):
                    eng.dma_start(
                        out=y[row : row + C_LOCAL, lo:hi],
                        in_=zsb[0:C_LOCAL, 0 : hi - lo],
                    ).then_inc(fill_sems[q], 16)
                    n_fills_q[q] += 1

            for k in gs[:PRIME]:
                copy(k)
            for i, k in enumerate(gs):
                if i + PRIME < len(gs):
                    copy(gs[i + PRIME])
                fill(k, first=(i == 0))

        # Terminal waits: the kernel may not retire until every DMA landed.
        for q, eng in enumerate(engines):
            for k in qgroups[q]:
                eng.wait_ge(copy_sems[k], 16)
            eng.wait_ge(fill_sems[q], 16 * n_fills_q[q] + 1)

    return nc


def _get_program(starts: np.ndarray, widths: np.ndarray) -> bass.Bass:
    key = starts.tobytes() + widths.tobytes()
    prog = _program_cache.get(key)
    if prog is None:
        prog = _build_program(_merged_windows(starts, widths))
        _program_cache[key] = prog
    return prog


def _run(x, starts, widths, trace=False, tmpdir=None):
    x = np.ascontiguousarray(x, dtype=np.float32)
    starts = np.asarray(starts, dtype=np.int32)
    widths = np.asarray(widths, dtype=np.int32)
    assert x.shape == (B, C, T), x.shape

    nc = _get_program(starts, widths)
    in_maps = [
        {
            "x": np.ascontiguousarray(
                x[:, k * C_LOCAL : (k + 1) * C_LOCAL, :]
            ).reshape(P, T)
        }
        for k in range(N_CORES)
    ]
    res = run_bass_kernel_spmd(
        nc, in_maps, list(range(N_CORES)), trace=trace, tmpdir=tmpdir
    )

    out = np.empty_like(x)
    for k in range(N_CORES):
        out[:, k * C_LOCAL : (k + 1) * C_LOCAL, :] = res.results[k]["y"].reshape(
            B, C_LOCAL, T
        )
    return out, res


def kernel(x, starts, widths):
    out, _ = _run(x, starts, widths, trace=False)
    return out


# revision 8
# speedup vs baseline: 1.0004x; 1.0004x over previous
"""Trainium2 Bass kernel for GPUTimeMask: zero out per-batch time windows.

Semantics (matches reference):
    out = x.copy();  for m, b:  out[b, :, s[m,b] : s[m,b]+clip(w[m,b],1,150)] = 0

Strategy (v6 — DRAM->DRAM group streams + HWDGE fills + segmented finals):
  - Shard x along the CHANNEL axis: 16 channels -> 2 per core across 8 cores.
    Every core holds ALL 64 batch rows, so the (runtime-valued) mask windows
    live at identical local coordinates on every core -> one SPMD program
    with window offsets specialized in at build time.
  - DRAM->DRAM copies: each SDMA descriptor reads and writes HBM inline, so
    HBM runs duplex (~650 GB/s measured -> ~330 GB/s copy rate, set by the
    16 SDMA engines' read->write turnaround).  No SBUF staging: that would
    pin the kernel to the ~435 GB/s SBUF-AXI fabric ceiling and cost two
    descriptor passes per byte instead of one.
  - The plane [128, 60000] f32 is split into 30 contiguous 4-row groups
    (full-width: 32 descriptors of 30 KB) round-robined over three issue
    queues: qSP + qAct (HWDGE) and the gpsimd SWDGE queue.
  - Mask windows are overwritten with zeros (from a memset SBUF tile) by
    tiny HWDGE DMAs ordered after the covering group's copy semaphore.
    Fills go ONLY on the HWDGE queues: a SWDGE fill expands to ~32 ring
    entries (vs ~3 for HWDGE) and head-of-line-blocks any copies behind it
    in the SWDGE ring, so gpsimd carries pure copies.  Fill runs alternate
    between qSP/qAct by group slot and are emitted in slot order between
    the engine's own copies, so each wait is (nearly) resolved when the
    sequencer reaches it and the ring never starves.
  - Each queue's FINAL group is special: its copies are split into the
    column segments BETWEEN its windows, and the windows are pre-filled
    with zeros early in the stream (copy and fill bytes are disjoint -> no
    ordering), so when the last copy descriptor lands there is NOTHING
    left but the terminal semaphore waits.
  - Raw bass (no TileContext).  Programs cached keyed on (starts, widths).
"""

import sys

import numpy as np

for _p in ("/opt/trn_rl_repo",):
    if _p not in sys.path:
        sys.path.insert(0, _p)

import concourse.bass as bass
import concourse.mybir as mybir
from concourse.bass_utils import run_bass_kernel_spmd

B, C, T = 64, 16, 60000
MAX_MASK_WIDTH = 150
N_CORES = 8
C_LOCAL = C // N_CORES          # 2 channels per core
P = B * C_LOCAL                 # 128 rows: row = b * C_LOCAL + c_local

GROUP_ROWS = 4
N_NORMAL = 30                   # 30 x 4-row normal groups = batches 0..59
PRIME = 3                       # own copies enqueued before the fill chase
SEG_CHUNK = 7500                # ragged segment split: n*7500 main + rem

_program_cache: dict[bytes, bass.Bass] = {}


def _merged_windows(starts: np.ndarray, widths: np.ndarray) -> list[list[tuple[int, int]]]:
    """Per-batch union of mask intervals (merge overlapping/adjacent)."""
    w = np.clip(widths, 1, MAX_MASK_WIDTH)
    out: list[list[tuple[int, int]]] = []
    for b in range(B):
        ivs = sorted(
            (int(starts[m, b]), min(int(starts[m, b]) + int(w[m, b]), T))
            for m in range(starts.shape[0])
        )
        merged = [ivs[0]]
        for s, e in ivs[1:]:
            if s <= merged[-1][1]:
                merged[-1] = (merged[-1][0], max(merged[-1][1], e))
            else:
                merged.append((s, e))
        out.append([(s, e) for s, e in merged if s < e])
    return out


def _build_program(windows: list[list[tuple[int, int]]]) -> bass.Bass:
    """windows[b]: merged (lo, hi) column ranges to zero; identical per core."""
    nc = bass.Bass()
    x = nc.declare_dram_parameter("x", [P, T], mybir.dt.float32, isOutput=False)
    y = nc.declare_dram_parameter("y", [P, T], mybir.dt.float32, isOutput=True)

    engines = [nc.sync, nc.scalar, nc.gpsimd]

    # Normal groups: slot s covers rows [4s, 4s+4) = batches 2s, 2s+1;
    # owner queue s%3.  Final groups: qSP batches 60,61 / qAct 62,63 —
    # wait: keep 4 batches for the two HWDGE finals (2 each) and give
    # gpsimd a final too by splitting differently below.
    #   rows 0..119   -> 30 normal groups (batches 0..59)
    #   batches 60,61 -> final group of qSP and qAct (one batch each)
    #   batches 62,63 -> final group of gpsimd (two batches)
    final_batches = {0: [60], 1: [61], 2: [62, 63]}

    copy_sems = [nc.alloc_semaphore(f"copy_s{s}") for s in range(N_NORMAL)]
    final_sems = [nc.alloc_semaphore(f"final_q{q}") for q in range(3)]
    fill_sems = [nc.alloc_semaphore(f"fills_q{q}") for q in range(2)]
    hs = nc.alloc_semaphore("zeros_ready")

    own = [[s for s in range(N_NORMAL) if s % 3 == q] for q in range(3)]
    n_fills = [0, 0]
    n_final_dmas = [0, 0, 0]

    with nc.sbuf_tensor("zeros", [32, MAX_MASK_WIDTH + 2], mybir.dt.float32) as zsb:
        nc.gpsimd.memset(zsb[:], 0.0)
        nc.gpsimd.sem_inc(hs, 1)

        def copy_group(q, s):
            r0, r1 = s * GROUP_ROWS, (s + 1) * GROUP_ROWS
            engines[q].dma_start(
                out=y[r0:r1, :], in_=x[r0:r1, :], max_dma_last_dim=30720
            ).then_inc(copy_sems[s], 16)

        def fill(q, row, lo, hi):
            engines[q].dma_start(
                out=y[row : row + C_LOCAL, lo:hi],
                in_=zsb[0:C_LOCAL, 0 : hi - lo],
            ).then_inc(fill_sems[q], 16)
            n_fills[q] += 1

        def seg_copy(q, b):
            """Copy batch b skipping its windows; manual 7500-col chunking
            keeps every descriptor a sane size for any segment length."""
            row = b * C_LOCAL
            segs, prev = [], 0
            for lo, hi in windows[b]:
                if lo > prev:
                    segs.append((prev, lo))
                prev = hi
            if prev < T:
                segs.append((prev, T))
            for lo, hi in segs:
                L = hi - lo
                n = L // SEG_CHUNK
                if n > 0:
                    engines[q].dma_start(
                        out=y[row : row + C_LOCAL, lo : lo + n * SEG_CHUNK],
                        in_=x[row : row + C_LOCAL, lo : lo + n * SEG_CHUNK],
                        max_dma_last_dim=30720,
                    ).then_inc(final_sems[q], 16)
                    n_final_dmas[q] += 1
                if lo + n * SEG_CHUNK < hi:
                    engines[q].dma_start(
                        out=y[row : row + C_LOCAL, lo + n * SEG_CHUNK : hi],
                        in_=x[row : row + C_LOCAL, lo + n * SEG_CHUNK : hi],
                    ).then_inc(final_sems[q], 16)
                    n_final_dmas[q] += 1

        # ---- gpsimd: pure copy stream, no waits anywhere ----
        for s in own[2]:
            copy_group(2, s)
        for b in final_batches[2]:
            seg_copy(2, b)

        # ---- HWDGE queues: primed copies, prefills, paced fill chase ----
        for q in (0, 1):
            eng = engines[q]
            for s in own[q][:PRIME]:
                copy_group(q, s)
            eng.wait_ge(hs, 1)
            # prefills for every final group's windows (disjoint from the
            # segmented copies -> unordered).  Split: q fills its own final
            # batch plus one of gpsimd's two.
            for b in final_batches[q] + [final_batches[2][q]]:
                for lo, hi in windows[b]:
                    fill(q, b * C_LOCAL, lo, hi)
            ci = PRIME
            for s in range(N_NORMAL):
                if s % 3 == q and ci < len(own[q]):
                    copy_group(q, own[q][ci])
                    ci += 1
                if s % 2 == q:
                    eng.wait_ge(copy_sems[s], 16)
                    for b in (2 * s, 2 * s + 1):
                        for lo, hi in windows[b]:
                            fill(q, b * C_LOCAL, lo, hi)
            for b in final_batches[q]:
                seg_copy(q, b)

        # ---- terminal completion waits ----
        for q in range(3):
            for s in own[q]:
                engines[q].wait_ge(copy_sems[s], 16)
            engines[q].wait_ge(final_sems[q], 16 * n_final_dmas[q])
            if q < 2:
                engines[q].wait_ge(fill_sems[q], 16 * n_fills[q])

    return nc


def _get_program(starts: np.ndarray, widths: np.ndarray) -> bass.Bass:
    key = starts.tobytes() + widths.tobytes()
    prog = _program_cache.get(key)
    if prog is None:
        prog = _build_program(_merged_windows(starts, widths))
        _program_cache[key] = prog
    return prog


def _run(x, starts, widths, trace=False, tmpdir=None):
    x = np.ascontiguousarray(x, dtype=np.float32)
    starts = np.asarray(starts, dtype=np.int32)
    widths = np.asarray(widths, dtype=np.int32)
    assert x.shape == (B, C, T), x.shape

    nc = _get_program(starts, widths)
    in_maps = [
        {
            "x": np.ascontiguousarray(
                x[:, k * C_LOCAL : (k + 1) * C_LOCAL, :]
            ).reshape(P, T)
        }
        for k in range(N_CORES)
    ]
    res = run_bass_kernel_spmd(
        nc, in_maps, list(range(N_CORES)), trace=trace, tmpdir=tmpdir
    )

    out = np.empty_like(x)
    for k in range(N_CORES):
        out[:, k * C_LOCAL : (k + 1) * C_LOCAL, :] = res.results[k]["y"].reshape(
            B, C_LOCAL, T
        )
    return out, res


def kernel(x, starts, widths):
    out, _ = _run(x, starts, widths, trace=False)
    return out


# revision 9
# speedup vs baseline: 1.0235x; 1.0232x over previous
"""Trainium2 Bass kernel for GPUTimeMask: zero out per-batch time windows.

Semantics (matches reference):
    out = x.copy();  for m, b:  out[b, :, s[m,b] : s[m,b]+clip(w[m,b],1,150)] = 0

Strategy (v7 — DRAM->DRAM group streams, per-queue fill interleave,
segmented tail-free finals):
  - Shard x along the CHANNEL axis: 16 channels -> 2 per core across 8
    cores.  Every core holds ALL 64 batch rows, so the (runtime-valued)
    mask windows live at identical local coordinates on every core -> one
    SPMD program with window offsets specialized in at build time.
  - DRAM->DRAM copies: each SDMA descriptor reads and writes HBM inline,
    so HBM runs duplex (~650 GB/s measured -> ~330 GB/s copy rate, set by
    the 16 SDMA engines' read->write turnaround).  No SBUF staging: that
    would pin the kernel to the ~435 GB/s SBUF-AXI fabric ceiling and
    cost two descriptor passes per byte instead of one.
  - 30 full-width 4-row groups stream over three queues (qSP + qAct HWDGE,
    gpsimd SWDGE).  Each queue interleaves ITS OWN groups' window fills
    behind its copy stream with a 3-group lag (v3 pattern): tiny fill
    descriptors cost ~0.15 us of ring time each, so they must alternate
    with big copy descriptors in the ring — bunching them (at the end, or
    on another queue's slot order) stalls the ring for tens of us.
  - gpsimd gets 8 of the 30 groups (its fills expand to ~32 ring entries
    vs ~3 for HWDGE, making it the slowest queue per byte).
  - The four remaining batches are tail-free FINALS, one set per queue:
    their copies skip the mask windows (segments split manually into
    n*7500-col chunks + remainder so every descriptor stays sane), and the
    windows are PRE-filled with zeros early in the stream.  Copy and fill
    bytes are disjoint -> no ordering -> when the last copy descriptor
    lands, nothing remains but the terminal semaphore waits.
  - Raw bass (no TileContext).  Programs cached keyed on (starts, widths).
"""

import sys

import numpy as np

for _p in ("/opt/trn_rl_repo",):
    if _p not in sys.path:
        sys.path.insert(0, _p)

import concourse.bass as bass
import concourse.mybir as mybir
from concourse.bass_utils import run_bass_kernel_spmd

B, C, T = 64, 16, 60000
MAX_MASK_WIDTH = 150
N_CORES = 8
C_LOCAL = C // N_CORES          # 2 channels per core
P = B * C_LOCAL                 # 128 rows: row = b * C_LOCAL + c_local

GROUP_ROWS = 4
N_NORMAL = 30                   # batches 0..59; finals are batches 60..63
PRIME = 3
SEG_CHUNK = 7500

_program_cache: dict[bytes, bass.Bass] = {}


def _owner(s: int) -> int:
    """Queue owning normal slot s: round-robin, two gpsimd slots donated
    to the HWDGE queues (gpsimd is the slowest per byte)."""
    if s == 23:
        return 0
    if s == 29:
        return 1
    return s % 3


_FINAL_BATCHES = {0: [60], 1: [61], 2: [62, 63]}


def _merged_windows(starts: np.ndarray, widths: np.ndarray) -> list[list[tuple[int, int]]]:
    """Per-batch union of mask intervals (merge overlapping/adjacent)."""
    w = np.clip(widths, 1, MAX_MASK_WIDTH)
    out: list[list[tuple[int, int]]] = []
    for b in range(B):
        ivs = sorted(
            (int(starts[m, b]), min(int(starts[m, b]) + int(w[m, b]), T))
            for m in range(starts.shape[0])
        )
        merged = [ivs[0]]
        for s, e in ivs[1:]:
            if s <= merged[-1][1]:
                merged[-1] = (merged[-1][0], max(merged[-1][1], e))
            else:
                merged.append((s, e))
        out.append([(s, e) for s, e in merged if s < e])
    return out


def _build_program(windows: list[list[tuple[int, int]]]) -> bass.Bass:
    """windows[b]: merged (lo, hi) column ranges to zero; identical per core."""
    nc = bass.Bass()
    x = nc.declare_dram_parameter("x", [P, T], mybir.dt.float32, isOutput=False)
    y = nc.declare_dram_parameter("y", [P, T], mybir.dt.float32, isOutput=True)

    engines = [nc.sync, nc.scalar, nc.gpsimd]
    copy_sems = [nc.alloc_semaphore(f"copy_s{s}") for s in range(N_NORMAL)]
    qsems = [nc.alloc_semaphore(f"qtail_{q}") for q in range(3)]
    hs = nc.alloc_semaphore("zeros_ready")

    own = [[s for s in range(N_NORMAL) if _owner(s) == q] for q in range(3)]
    n_tail = [0, 0, 0]  # per-queue count of fills + seg-copy dmas on qsems

    with nc.sbuf_tensor("zeros", [32, MAX_MASK_WIDTH + 2], mybir.dt.float32) as zsb:
        nc.gpsimd.memset(zsb[:], 0.0)
        nc.gpsimd.sem_inc(hs, 1)

        def copy_group(q, s):
            r0, r1 = s * GROUP_ROWS, (s + 1) * GROUP_ROWS
            engines[q].dma_start(
                out=y[r0:r1, :], in_=x[r0:r1, :], max_dma_last_dim=30720
            ).then_inc(copy_sems[s], 16)

        def fill(q, b):
            for lo, hi in windows[b]:
                engines[q].dma_start(
                    out=y[b * C_LOCAL : b * C_LOCAL + C_LOCAL, lo:hi],
                    in_=zsb[0:C_LOCAL, 0 : hi - lo],
                ).then_inc(qsems[q], 16)
                n_tail[q] += 1

        def seg_copy(q, b):
            """Copy batch b skipping its windows; manual chunking keeps
            every descriptor a sane size for any segment length."""
            row = b * C_LOCAL
            segs, prev = [], 0
            for lo, hi in windows[b]:
                if lo > prev:
                    segs.append((prev, lo))
                prev = hi
            if prev < T:
                segs.append((prev, T))
            for lo, hi in segs:
                n = (hi - lo) // SEG_CHUNK
                for a, bnd in ((lo, lo + n * SEG_CHUNK), (lo + n * SEG_CHUNK, hi)):
                    if a < bnd:
                        engines[q].dma_start(
                            out=y[row : row + C_LOCAL, a:bnd],
                            in_=x[row : row + C_LOCAL, a:bnd],
                            max_dma_last_dim=30720,
                        ).then_inc(qsems[q], 16)
                        n_tail[q] += 1

        for q in range(3):
            eng = engines[q]
            for s in own[q][:PRIME]:
                copy_group(q, s)
            if q != 2:
                eng.wait_ge(hs, 1)
            # unordered prefills for this queue's tail-free final batches
            for b in _FINAL_BATCHES[q]:
                fill(q, b)
            # v3 interleave: copy own[i+PRIME], then chase own[i]'s fills
            for i, s in enumerate(own[q]):
                if i + PRIME < len(own[q]):
                    copy_group(q, own[q][i + PRIME])
                eng.wait_ge(copy_sems[s], 16)
                for b in (2 * s, 2 * s + 1):
                    fill(q, b)
            # segmented finals: last bytes in the ring, nothing after them
            for b in _FINAL_BATCHES[q]:
                seg_copy(q, b)

        for q in range(3):
            for s in own[q]:
                engines[q].wait_ge(copy_sems[s], 16)
            engines[q].wait_ge(qsems[q], 16 * n_tail[q])

    return nc


def _get_program(starts: np.ndarray, widths: np.ndarray) -> bass.Bass:
    key = starts.tobytes() + widths.tobytes()
    prog = _program_cache.get(key)
    if prog is None:
        prog = _build_program(_merged_windows(starts, widths))
        _program_cache[key] = prog
    return prog


def _run(x, starts, widths, trace=False, tmpdir=None):
    x = np.ascontiguousarray(x, dtype=np.float32)
    starts = np.asarray(starts, dtype=np.int32)
    widths = np.asarray(widths, dtype=np.int32)
    assert x.shape == (B, C, T), x.shape

    nc = _get_program(starts, widths)
    in_maps = [
        {
            "x": np.ascontiguousarray(
                x[:, k * C_LOCAL : (k + 1) * C_LOCAL, :]
            ).reshape(P, T)
        }
        for k in range(N_CORES)
    ]
    res = run_bass_kernel_spmd(
        nc, in_maps, list(range(N_CORES)), trace=trace, tmpdir=tmpdir
    )

    out = np.empty_like(x)
    for k in range(N_CORES):
        out[:, k * C_LOCAL : (k + 1) * C_LOCAL, :] = res.results[k]["y"].reshape(
            B, C_LOCAL, T
        )
    return out, res


def kernel(x, starts, widths):
    out, _ = _run(x, starts, widths, trace=False)
    return out
